# revision 31
# baseline (speedup 1.0000x reference)
"""Trainium2 Bass kernel for nn_Decoder_78486232367391.

GRU decoder: 63 sequential recurrence steps + a large vocab unembed.
Strategy:
  - Phase 1 (recurrence): computed redundantly on all 8 cores in
    (feature-on-partitions, batch-on-free) layout; embeddings for every
    timestep are accumulated into an internal DRAM buffer.
  - Phase 2 (unembed): vocab dimension (V=32000) sharded column-wise,
    4000 columns per core; batched over all T*B=3072 rows at full PE
    utilization. Host concatenates the 8 vocab slices.
Outputs match the jax reference: (outputs (B,T,V), stops (B,T,1)).
"""

import numpy as np

import concourse.bass as bass
import concourse.mybir as mybir
import concourse.tile as tile
from concourse import bacc
from concourse.bass_utils import run_bass_kernel_spmd

B = 48
E = 512
H = 1024
V = 32000
T = 64
NCORES = 8
VS = V // NCORES  # vocab slice per core

F32 = mybir.dt.float32
BF16 = mybir.dt.bfloat16
F32R = mybir.dt.float32r

import os as _os
# matmul precision knobs ("bf16" | "f32" | "f32r"); host prep must agree
GATES_DT = _os.environ.get("K_GATES_DT", "bf16")     # recurrence weight matmuls
UNEMBED_DT = _os.environ.get("K_UNEMBED_DT", "bf16")  # vocab projection

_DT = {"bf16": BF16, "f32": F32, "f32r": F32R}

_CACHE = {}


def _bc(dram, ap):
    """Broadcast/strided view of a DRAM tensor via an explicit access pattern."""
    a = dram[:] if not isinstance(dram, bass.AP) else dram
    return bass.AP(tensor=a.tensor, offset=a.offset, ap=ap)


def build(t_steps=T):
    nc = bacc.Bacc("TRN2")
    GDT = _DT[GATES_DT]
    UDT = _DT[UNEMBED_DT]

    def inp(name, shape, dt=F32):
        return nc.declare_dram_parameter(name, list(shape), dt, isOutput=False)

    wrzT = inp("wrzT", (E + H, 2 * H), GDT)  # [emb;h] -> r,z rows
    winT = inp("winT", (E, H), GDT)          # emb -> i_n
    whnT = inp("whnT", (H, H), GDT)          # h -> h_n
    wpT = inp("wpT", (H, 3 * H))             # latent -> peep
    wlhT = inp("wlhT", (H, H))               # latent -> hidden0
    woT = inp("woT", (H, E), GDT)            # h -> o[:E]
    wuT = inp("wuT", (E, VS), UDT)           # per-core unembed slice
    latT = inp("latT", (H, B))
    e0T = inp("e0T", (E, B), GDT)
    # biases pre-broadcast on host into on-chip layouts
    brz_b = inp("brz_b", (128, 2 * H // 128, B))   # bi+bh+bp over rz rows
    bnc_b = inp("bnc_b", (128, H // 128, B))       # bi+bp over n rows
    bhn_b = inp("bhn_b", (128, H // 128, B))       # bh over n rows
    blh_b = inp("blh_b", (128, H // 128, B))
    bo_p = inp("bo_p", (128, E // 128))
    bo_s = inp("bo_s", (1, 1))
    wos_p = inp("wos_p", (128, H // 128), GDT)     # stop-row weights per k-tile
    bu_b = inp("bu_b", (128, VS))
    assert GATES_DT == UNEMBED_DT  # ebuf feeds phase 2 directly

    logits = nc.declare_dram_parameter("logits", [t_steps * B, VS], F32, isOutput=True)
    stops = nc.declare_dram_parameter("stops", [1, t_steps * B], F32, isOutput=True)

    ebuf = nc.dram_tensor("ebuf", [E, t_steps * B], UDT)

    KRZ = (E + H) // 128   # 12 contraction tiles for rz
    KE = E // 128          # 4
    KH = H // 128          # 8
    MRZ = 2 * H // 128     # 16 gate row tiles
    MH = H // 128          # 8
    ME = E // 128          # 4

    with tile.TileContext(nc) as tc:
        import contextlib
        with contextlib.ExitStack() as stack:
            consts = stack.enter_context(tc.tile_pool(name="consts", bufs=1))
            # broadcast bias matrices (value per (partition, slot), repeated over batch)
            bhn_bc = consts.tile([128, MH, B], F32)
            nc.sync.dma_start(out=bhn_bc, in_=bhn_b[:, :, :])
            bo_sb = consts.tile([128, ME], F32)
            nc.sync.dma_start(out=bo_sb, in_=bo_p[:, :])
            bo_s_sb = consts.tile([1, 1], F32)
            nc.sync.dma_start(out=bo_s_sb, in_=bo_s[:, :])
            c_rz = consts.tile([128, MRZ, B], F32)
            c_n = consts.tile([128, MH, B], F32)
            e0_sb = consts.tile([128, KE, B], GDT)
            h0_sb = consts.tile([128, KH, B], F32)
            h0_bf = consts.tile([128, KH, B], GDT)

            # ---- pre-step: peep gates (c_rz, c_n) and hidden0, streamed ----
            with tc.tile_pool(name="prew", bufs=6) as prew, \
                 tc.tile_pool(name="prec", bufs=1) as prec, \
                 tc.tile_pool(name="prepsum", bufs=1, space="PSUM") as prepsum:
                brz_bc = prec.tile([128, MRZ, B], F32)
                nc.sync.dma_start(out=brz_bc, in_=brz_b[:, :, :])
                bnc_bc = prec.tile([128, MH, B], F32)
                nc.sync.dma_start(out=bnc_bc, in_=bnc_b[:, :, :])
                blh_bc = prec.tile([128, MH, B], F32)
                nc.sync.dma_start(out=blh_bc, in_=blh_b[:, :, :])
                lat_sb = prec.tile([128, KH, B], F32)
                for k in range(KH):
                    nc.sync.dma_start(
                        out=lat_sb[:, k, :], in_=latT[k * 128:(k + 1) * 128, :])

                p_pre = prepsum.tile([128, 3 * MH, 64], F32)   # 24 slots: rz then n
                for m in range(3 * MH):
                    for k in range(KH):
                        wp_blk = prew.tile([128, 128], F32, tag="wp")
                        nc.sync.dma_start(
                            out=wp_blk,
                            in_=wpT[k * 128:(k + 1) * 128, m * 128:(m + 1) * 128])
                        nc.tensor.matmul(
                            p_pre[:, m, 0:B], wp_blk, lat_sb[:, k, :],
                            start=(k == 0), stop=(k == KH - 1),
                        )
                nc.vector.tensor_add(c_rz, p_pre[:, 0:MRZ, 0:B], brz_bc)
                nc.vector.tensor_add(c_n, p_pre[:, MRZ:3 * MH, 0:B], bnc_bc)

                p_h0 = prepsum.tile([128, MH, 64], F32)
                for m in range(MH):
                    for k in range(KH):
                        wlh_blk = prew.tile([128, 128], F32, tag="wp")
                        nc.sync.dma_start(
                            out=wlh_blk,
                            in_=wlhT[k * 128:(k + 1) * 128, m * 128:(m + 1) * 128])
                        nc.tensor.matmul(
                            p_h0[:, m, 0:B], wlh_blk, lat_sb[:, k, :],
                            start=(k == 0), stop=(k == KH - 1),
                        )
                nc.vector.tensor_add(h0_sb, p_h0[:, :, 0:B], blh_bc)
                nc.vector.tensor_copy(h0_bf, h0_sb)

                # initial embedding (start token) + stop_0 = 0
                for k in range(KE):
                    nc.sync.dma_start(
                        out=e0_sb[:, k, :], in_=e0T[k * 128:(k + 1) * 128, :])
                    nc.sync.dma_start(
                        out=ebuf[k * 128:(k + 1) * 128, 0:B], in_=e0_sb[:, k, :])
                s0 = prec.tile([1, B], F32)
                nc.vector.memset(s0, 0.0)
                nc.sync.dma_start(out=stops[:, 0:B], in_=s0)

            # ---- recurrence weights (resident) ----
            wpool = stack.enter_context(tc.tile_pool(name="rweights", bufs=1))
            wrz_sb = wpool.tile([128, KRZ, 2 * H], GDT)
            for k in range(KRZ):
                nc.sync.dma_start(out=wrz_sb[:, k, :], in_=wrzT[k * 128:(k + 1) * 128, :])
            win_sb = wpool.tile([128, KE, H], GDT)
            for k in range(KE):
                nc.sync.dma_start(out=win_sb[:, k, :], in_=winT[k * 128:(k + 1) * 128, :])
            whn_sb = wpool.tile([128, KH, H], GDT)
            for k in range(KH):
                nc.sync.dma_start(out=whn_sb[:, k, :], in_=whnT[k * 128:(k + 1) * 128, :])
            wo_sb = wpool.tile([128, KH, E], GDT)
            for k in range(KH):
                nc.sync.dma_start(out=wo_sb[:, k, :], in_=woT[k * 128:(k + 1) * 128, :])
            wos_sb = wpool.tile([128, KH], GDT)
            nc.sync.dma_start(out=wos_sb, in_=wos_p[:, :])

            # ---- main recurrence ----
            state = stack.enter_context(tc.tile_pool(name="state", bufs=2))
            gwork = stack.enter_context(tc.tile_pool(name="gwork", bufs=2))
            mmp = stack.enter_context(tc.tile_pool(name="mmp", bufs=2, space="PSUM"))
            mmp1 = stack.enter_context(tc.tile_pool(name="mmp1", bufs=1, space="PSUM"))

            e_prev, h_prev, h_prev_bf = e0_sb, h0_sb, h0_bf
            for t in range(1, t_steps):
                p_rz = mmp.tile([128, MRZ, 64], F32, tag="p_rz")
                p_n = mmp1.tile([128, 2 * MH, 64], F32, tag="p_n")
                p_o = mmp.tile([128, MH, 64], F32, tag="p_o")

                def xcat(k):
                    return e_prev[:, k, :] if k < KE else h_prev_bf[:, k - KE, :]

                for m in range(MRZ):
                    for k in range(KRZ):
                        nc.tensor.matmul(
                            p_rz[:, m, 0:B],
                            wrz_sb[:, k, m * 128:(m + 1) * 128],
                            xcat(k),
                            start=(k == 0), stop=(k == KRZ - 1),
                        )
                for m in range(MH):
                    for k in range(KE):
                        nc.tensor.matmul(
                            p_n[:, m, 0:B],
                            win_sb[:, k, m * 128:(m + 1) * 128],
                            e_prev[:, k, :],
                            start=(k == 0), stop=(k == KE - 1),
                        )
                for m in range(MH):
                    for k in range(KH):
                        nc.tensor.matmul(
                            p_n[:, MH + m, 0:B],
                            whn_sb[:, k, m * 128:(m + 1) * 128],
                            h_prev_bf[:, k, :],
                            start=(k == 0), stop=(k == KH - 1),
                        )

                s_rz = gwork.tile([128, MRZ, B], F32, tag="s_rz")
                nc.vector.tensor_add(s_rz, p_rz[:, :, 0:B], c_rz)
                g_rz = gwork.tile([128, MRZ, B], F32, tag="g_rz")
                nc.scalar.activation(
                    out=g_rz, in_=s_rz, func=mybir.ActivationFunctionType.Sigmoid)
                r = g_rz[:, 0:MH, :]
                z = g_rz[:, MH:MRZ, :]

                s_hn = gwork.tile([128, MH, B], F32, tag="s_hn")
                nc.vector.tensor_add(s_hn, p_n[:, MH:2 * MH, 0:B], bhn_bc)
                t1 = gwork.tile([128, MH, B], F32, tag="t1")
                nc.vector.tensor_mul(t1, r, s_hn)
                nc.vector.tensor_add(t1, t1, p_n[:, 0:MH, 0:B])
                nc.vector.tensor_add(t1, t1, c_n)
                n_g = gwork.tile([128, MH, B], F32, tag="n_g")
                nc.scalar.activation(
                    out=n_g, in_=t1, func=mybir.ActivationFunctionType.Tanh)

                d = gwork.tile([128, MH, B], F32, tag="d")
                nc.vector.tensor_sub(d, h_prev, n_g)
                nc.vector.tensor_mul(d, z, d)
                h_new = state.tile([128, MH, B], F32, tag="h")
                nc.vector.tensor_add(h_new, n_g, d)
                h_new_bf = state.tile([128, MH, B], GDT, tag="hbf")
                nc.vector.tensor_copy(h_new_bf, h_new)

                for m in range(ME):
                    for k in range(KH):
                        nc.tensor.matmul(
                            p_o[:, m, 0:B],
                            wo_sb[:, k, m * 128:(m + 1) * 128],
                            h_new_bf[:, k, :],
                            start=(k == 0), stop=(k == KH - 1),
                        )
                for k in range(KH):
                    nc.tensor.matmul(
                        p_o[0:1, ME, 0:B],
                        wos_sb[:, k:k + 1],
                        h_new_bf[:, k, :],
                        start=(k == 0), stop=(k == KH - 1),
                    )

                e_new = state.tile([128, KE, B], GDT, tag="e")
                for m in range(ME):
                    nc.scalar.activation(
                        out=e_new[:, m, :], in_=p_o[:, m, 0:B],
                        func=mybir.ActivationFunctionType.Tanh,
                        bias=bo_sb[:, m:m + 1])
                st = gwork.tile([1, B], F32, tag="st")
                nc.scalar.activation(
                    out=st, in_=p_o[0:1, ME, 0:B],
                    func=mybir.ActivationFunctionType.Sigmoid,
                    bias=bo_s_sb[0:1, 0:1])
                nc.sync.dma_start(out=stops[:, t * B:(t + 1) * B], in_=st)
                for k in range(KE):
                    nc.sync.dma_start(
                        out=ebuf[k * 128:(k + 1) * 128, t * B:(t + 1) * B],
                        in_=e_new[:, k, :])

                e_prev, h_prev, h_prev_bf = e_new, h_new, h_new_bf

        # ---- phase 2: batched unembed over the vocab slice ----
        tc.strict_bb_all_engine_barrier()

        MT = t_steps * B // 128  # row tiles over (t, b)
        NT = (VS + 511) // 512
        with tc.tile_pool(name="wu", bufs=1) as wup, \
             tc.tile_pool(name="p2in", bufs=3) as p2in, \
             tc.tile_pool(name="p2out", bufs=4) as p2out, \
             tc.tile_pool(name="p2ps", bufs=2, space="PSUM") as p2ps:
            wu_sb = wup.tile([128, KE, VS], UDT)
            for k in range(KE):
                nc.sync.dma_start(out=wu_sb[:, k, :], in_=wuT[k * 128:(k + 1) * 128, :])
            bu_bc = wup.tile([128, VS], F32)
            nc.sync.dma_start(out=bu_bc, in_=bu_b[:, :])

            for m in range(MT):
                lhs_m = p2in.tile([128, KE, 128], UDT, tag="lhs")
                for k in range(KE):
                    nc.sync.dma_start(
                        out=lhs_m[:, k, :],
                        in_=ebuf[k * 128:(k + 1) * 128, m * 128:(m + 1) * 128])
                for n in range(NT):
                    nw = min(512, VS - n * 512)
                    p2 = p2ps.tile([128, 512], F32, tag="p2")
                    for k in range(KE):
                        nc.tensor.matmul(
                            p2[:, 0:nw],
                            lhs_m[:, k, :],
                            wu_sb[:, k, n * 512:n * 512 + nw],
                            start=(k == 0), stop=(k == KE - 1),
                        )
                    o_t = p2out.tile([128, 512], F32, tag="o")
                    nc.vector.tensor_add(
                        o_t[:, 0:nw], p2[:, 0:nw], bu_bc[:, n * 512:n * 512 + nw])
                    nc.sync.dma_start(
                        out=logits[m * 128:(m + 1) * 128, n * 512:n * 512 + nw],
                        in_=o_t[:, 0:nw])

    nc.compile()
    return nc


def _prep_inputs(inputs):
    f = lambda x: np.ascontiguousarray(np.asarray(x, dtype=np.float32))
    Wi, Wh, Wp = f(inputs["Wi"]), f(inputs["Wh"]), f(inputs["Wp"])
    bi, bh, bp = f(inputs["bi"]), f(inputs["bh"]), f(inputs["bp"])
    Wo, bo = f(inputs["Wo"]), f(inputs["bo"])
    Wu, bu = f(inputs["Wu"]), f(inputs["bu"])
    W_lh, b_lh = f(inputs["W_lh"]), f(inputs["b_lh"])
    latent, start = f(inputs["latent"]), f(inputs["start"])

    import ml_dtypes
    _npdt = {"bf16": ml_dtypes.bfloat16, "f32": np.float32, "f32r": np.float32}
    gdt = _npdt[GATES_DT]
    udt = _npdt[UNEMBED_DT]

    def pb(v, slots):
        # (slots*128,) -> (128, slots, B) broadcast over batch
        return np.ascontiguousarray(
            np.repeat(v.reshape(slots, 128).T[:, :, None], B, axis=2))

    base = {
        "wrzT": np.ascontiguousarray(
            np.concatenate([Wi[:2 * H], Wh[:2 * H]], axis=1).T).astype(gdt),
        "winT": np.ascontiguousarray(Wi[2 * H:].T).astype(gdt),
        "whnT": np.ascontiguousarray(Wh[2 * H:].T).astype(gdt),
        "wpT": np.ascontiguousarray(Wp.T),
        "wlhT": np.ascontiguousarray(W_lh.T),
        "woT": np.ascontiguousarray(Wo[:E].T).astype(gdt),
        "latT": np.ascontiguousarray(latent.T),
        "e0T": np.ascontiguousarray(
            np.repeat(start[:, None], B, axis=1)).astype(gdt),
        "brz_b": pb((bi + bh + bp)[:2 * H], 16),
        "bnc_b": pb((bi + bp)[2 * H:], 8),
        "bhn_b": pb(bh[2 * H:], 8),
        "blh_b": pb(b_lh, 8),
        "bo_p": np.ascontiguousarray(bo[:E].reshape(E // 128, 128).T),
        "bo_s": bo[E:E + 1].reshape(1, 1),
        "wos_p": np.ascontiguousarray(Wo[E].reshape(H // 128, 128).T).astype(gdt),
    }
    in_maps = []
    for c in range(NCORES):
        m = dict(base)
        m["wuT"] = np.ascontiguousarray(Wu[c * VS:(c + 1) * VS].T).astype(udt)
        m["bu_b"] = np.ascontiguousarray(
            np.repeat(bu[None, c * VS:(c + 1) * VS], 128, axis=0))
        in_maps.append(m)
    return in_maps


def kernel(**inputs):
    if "nc" not in _CACHE:
        _CACHE["nc"] = build()
    nc = _CACHE["nc"]
    in_maps = _prep_inputs(inputs)
    res = run_bass_kernel_spmd(nc, in_maps, list(range(NCORES)))
    outs = [res.results[c]["logits"] for c in range(NCORES)]
    logits = np.concatenate(outs, axis=1).reshape(T, B, V).transpose(1, 0, 2)
    stops = res.results[0]["stops"].reshape(T, B).T[:, :, None]
    return np.ascontiguousarray(logits), np.ascontiguousarray(stops)


# revision 37
# speedup vs baseline: 2.1065x; 2.1065x over previous
"""Trainium2 Bass kernel for nn_Decoder_78486232367391.

GRU decoder: 63 sequential recurrence steps + a large vocab unembed.
Strategy:
  - Phase 1 (recurrence): computed redundantly on all 8 cores in
    (feature-on-partitions, batch-on-free) layout; embeddings for every
    timestep are accumulated into an internal DRAM buffer.
  - Phase 2 (unembed): vocab dimension (V=32000) sharded column-wise,
    4000 columns per core; batched over all T*B=3072 rows at full PE
    utilization. Host concatenates the 8 vocab slices.
Outputs match the jax reference: (outputs (B,T,V), stops (B,T,1)).
"""

import numpy as np

import concourse.bass as bass
import concourse.mybir as mybir
import concourse.tile as tile
from concourse import bacc
from concourse.bass_utils import run_bass_kernel_spmd

B = 48
E = 512
H = 1024
V = 32000
T = 64
NCORES = 8
VS = V // NCORES  # vocab slice per core

F32 = mybir.dt.float32
BF16 = mybir.dt.bfloat16
F32R = mybir.dt.float32r

import os as _os
# matmul precision knobs ("bf16" | "f32" | "f32r"); host prep must agree
GATES_DT = _os.environ.get("K_GATES_DT", "bf16")     # recurrence weight matmuls
UNEMBED_DT = _os.environ.get("K_UNEMBED_DT", "bf16")  # vocab projection

_DT = {"bf16": BF16, "f32": F32, "f32r": F32R}

_CACHE = {}


def _bc(dram, ap):
    """Broadcast/strided view of a DRAM tensor via an explicit access pattern."""
    a = dram[:] if not isinstance(dram, bass.AP) else dram
    return bass.AP(tensor=a.tensor, offset=a.offset, ap=ap)


def build(t_steps=T):
    nc = bacc.Bacc("TRN2")
    GDT = _DT[GATES_DT]
    UDT = _DT[UNEMBED_DT]

    def inp(name, shape, dt=F32):
        return nc.declare_dram_parameter(name, list(shape), dt, isOutput=False)

    wrzT = inp("wrzT", (E + H, 2 * H), GDT)  # [emb;h] -> r,z rows
    winT = inp("winT", (E, H), GDT)          # emb -> i_n
    whnT = inp("whnT", (H, H), GDT)          # h -> h_n
    wpT = inp("wpT", (H, 3 * H))             # latent -> peep
    wlhT = inp("wlhT", (H, H))               # latent -> hidden0
    woT = inp("woT", (H, E), GDT)            # h -> o[:E]
    wuT = inp("wuT", (E, VS), UDT)           # per-core unembed slice
    latT = inp("latT", (H, B))
    e0T = inp("e0T", (E, B), GDT)
    # biases pre-broadcast on host into on-chip layouts
    brz_b = inp("brz_b", (128, 2 * H // 128, B))   # bi+bh+bp over rz rows
    bnc_b = inp("bnc_b", (128, H // 128, B))       # bi+bp over n rows
    bhn_b = inp("bhn_b", (128, H // 128, B))       # bh over n rows
    blh_b = inp("blh_b", (128, H // 128, B))
    bo_p = inp("bo_p", (128, E // 128))
    bo_s = inp("bo_s", (1, 1))
    wos_p = inp("wos_p", (128, H // 128), GDT)     # stop-row weights per k-tile
    bu_b = inp("bu_b", (128, VS))
    assert GATES_DT == UNEMBED_DT  # ebuf feeds phase 2 directly

    logits = nc.declare_dram_parameter("logits", [t_steps * B, VS], F32, isOutput=True)
    stops = nc.declare_dram_parameter("stops", [1, t_steps * B], F32, isOutput=True)

    assert t_steps % 8 == 0  # unembed runs on groups of 8 steps (384 rows)

    KRZ = (E + H) // 128   # 12 contraction tiles for rz
    KE = E // 128          # 4
    KH = H // 128          # 8
    MRZ = 2 * H // 128     # 16 gate row tiles
    MH = H // 128          # 8
    ME = E // 128          # 4

    with tile.TileContext(nc) as tc:
        import contextlib
        with contextlib.ExitStack() as stack:
            consts = stack.enter_context(tc.tile_pool(name="consts", bufs=1))
            # broadcast bias matrices (value per (partition, slot), repeated over batch)
            bhn_bc = consts.tile([128, MH, B], F32)
            nc.sync.dma_start(out=bhn_bc, in_=bhn_b[:, :, :])
            bo_sb = consts.tile([128, ME], F32)
            nc.sync.dma_start(out=bo_sb, in_=bo_p[:, :])
            bo_s_sb = consts.tile([1, 1], F32)
            nc.sync.dma_start(out=bo_s_sb, in_=bo_s[:, :])
            c_rz = consts.tile([128, MRZ, B], F32)
            c_n = consts.tile([128, MH, B], F32)
            h0_sb = consts.tile([128, KH, B], F32)
            h0_bf = consts.tile([128, KH, B], GDT)
            # ring of the last 8 steps' embeddings, consumed by the unembed
            ering = consts.tile([128, KE, 8 * B], GDT)
            wu_sb = consts.tile([128, KE, VS], UDT)
            for k in range(KE):
                nc.sync.dma_start(out=wu_sb[:, k, :], in_=wuT[k * 128:(k + 1) * 128, :])
            bu_bc = consts.tile([128, VS], F32)
            nc.sync.dma_start(out=bu_bc, in_=bu_b[:, :])

            # ---- pre-step: peep gates (c_rz, c_n) and hidden0, streamed ----
            with tc.tile_pool(name="prew", bufs=6) as prew, \
                 tc.tile_pool(name="prec", bufs=1) as prec, \
                 tc.tile_pool(name="prepsum", bufs=1, space="PSUM") as prepsum:
                brz_bc = prec.tile([128, MRZ, B], F32)
                nc.sync.dma_start(out=brz_bc, in_=brz_b[:, :, :])
                bnc_bc = prec.tile([128, MH, B], F32)
                nc.sync.dma_start(out=bnc_bc, in_=bnc_b[:, :, :])
                blh_bc = prec.tile([128, MH, B], F32)
                nc.sync.dma_start(out=blh_bc, in_=blh_b[:, :, :])
                lat_sb = prec.tile([128, KH, B], F32)
                for k in range(KH):
                    nc.sync.dma_start(
                        out=lat_sb[:, k, :], in_=latT[k * 128:(k + 1) * 128, :])

                p_pre = prepsum.tile([128, 3 * MH, 64], F32)   # 24 slots: rz then n
                for m in range(3 * MH):
                    for k in range(KH):
                        wp_blk = prew.tile([128, 128], F32, tag="wp")
                        nc.sync.dma_start(
                            out=wp_blk,
                            in_=wpT[k * 128:(k + 1) * 128, m * 128:(m + 1) * 128])
                        nc.tensor.matmul(
                            p_pre[:, m, 0:B], wp_blk, lat_sb[:, k, :],
                            start=(k == 0), stop=(k == KH - 1),
                        )
                nc.vector.tensor_add(c_rz, p_pre[:, 0:MRZ, 0:B], brz_bc)
                nc.vector.tensor_add(c_n, p_pre[:, MRZ:3 * MH, 0:B], bnc_bc)

                p_h0 = prepsum.tile([128, MH, 64], F32)
                for m in range(MH):
                    for k in range(KH):
                        wlh_blk = prew.tile([128, 128], F32, tag="wp")
                        nc.sync.dma_start(
                            out=wlh_blk,
                            in_=wlhT[k * 128:(k + 1) * 128, m * 128:(m + 1) * 128])
                        nc.tensor.matmul(
                            p_h0[:, m, 0:B], wlh_blk, lat_sb[:, k, :],
                            start=(k == 0), stop=(k == KH - 1),
                        )
                nc.vector.tensor_add(h0_sb, p_h0[:, :, 0:B], blh_bc)
                nc.vector.tensor_copy(h0_bf, h0_sb)

                # initial embedding (start token) + stop_0 = 0
                for k in range(KE):
                    nc.sync.dma_start(
                        out=ering[:, k, 0:B], in_=e0T[k * 128:(k + 1) * 128, :])
                s0 = prec.tile([1, B], F32)
                nc.vector.memset(s0, 0.0)
                nc.sync.dma_start(out=stops[:, 0:B], in_=s0)

            # ---- recurrence weights (resident) ----
            wpool = stack.enter_context(tc.tile_pool(name="rweights", bufs=1))
            wrz_sb = wpool.tile([128, KRZ, 2 * H], GDT)
            for k in range(KRZ):
                nc.sync.dma_start(out=wrz_sb[:, k, :], in_=wrzT[k * 128:(k + 1) * 128, :])
            win_sb = wpool.tile([128, KE, H], GDT)
            for k in range(KE):
                nc.sync.dma_start(out=win_sb[:, k, :], in_=winT[k * 128:(k + 1) * 128, :])
            whn_sb = wpool.tile([128, KH, H], GDT)
            for k in range(KH):
                nc.sync.dma_start(out=whn_sb[:, k, :], in_=whnT[k * 128:(k + 1) * 128, :])
            wo_sb = wpool.tile([128, KH, E], GDT)
            for k in range(KH):
                nc.sync.dma_start(out=wo_sb[:, k, :], in_=woT[k * 128:(k + 1) * 128, :])
            wos_sb = wpool.tile([128, KH], GDT)
            nc.sync.dma_start(out=wos_sb, in_=wos_p[:, :])

            # ---- main recurrence with interleaved unembed groups ----
            state = stack.enter_context(tc.tile_pool(name="state", bufs=2))
            gwork = stack.enter_context(tc.tile_pool(name="gwork", bufs=2))
            p2out = stack.enter_context(tc.tile_pool(name="p2out", bufs=4))
            mmp = stack.enter_context(tc.tile_pool(name="mmp", bufs=1, space="PSUM"))
            mmp1 = stack.enter_context(tc.tile_pool(name="mmp1", bufs=1, space="PSUM"))
            p2ps = stack.enter_context(tc.tile_pool(name="p2ps", bufs=2, space="PSUM"))
            NT = (VS + 511) // 512

            def unembed_group(g):
                # steps 8g..8g+7 fill ering columns 0..383; emit 3 row tiles
                for j in range(3):
                    for n in range(NT):
                        nw = min(512, VS - n * 512)
                        p2 = p2ps.tile([128, 512], F32, tag="p2")
                        for k in range(KE):
                            nc.tensor.matmul(
                                p2[:, 0:nw],
                                ering[:, k, j * 128:(j + 1) * 128],
                                wu_sb[:, k, n * 512:n * 512 + nw],
                                start=(k == 0), stop=(k == KE - 1),
                            )
                        o_t = p2out.tile([128, 512], F32, tag="o")
                        nc.vector.tensor_add(
                            o_t[:, 0:nw], p2[:, 0:nw], bu_bc[:, n * 512:n * 512 + nw])
                        row = g * 384 + j * 128
                        nc.sync.dma_start(
                            out=logits[row:row + 128, n * 512:n * 512 + nw],
                            in_=o_t[:, 0:nw])

            def e_slot(t):
                return ering[:, :, (t % 8) * B:(t % 8 + 1) * B]

            e_prev, h_prev, h_prev_bf = e_slot(0), h0_sb, h0_bf
            for t in range(1, t_steps):
                p_rz = mmp.tile([128, MRZ, 64], F32, tag="p_rz")
                p_n = mmp1.tile([128, 2 * MH, 64], F32, tag="p_n")
                p_o = mmp.tile([128, MH, 64], F32, tag="p_o")

                def xcat(k):
                    return e_prev[:, k, :] if k < KE else h_prev_bf[:, k - KE, :]

                for m in range(MRZ):
                    for k in range(KRZ):
                        nc.tensor.matmul(
                            p_rz[:, m, 0:B],
                            wrz_sb[:, k, m * 128:(m + 1) * 128],
                            xcat(k),
                            start=(k == 0), stop=(k == KRZ - 1),
                        )
                for m in range(MH):
                    for k in range(KE):
                        nc.tensor.matmul(
                            p_n[:, m, 0:B],
                            win_sb[:, k, m * 128:(m + 1) * 128],
                            e_prev[:, k, :],
                            start=(k == 0), stop=(k == KE - 1),
                        )
                for m in range(MH):
                    for k in range(KH):
                        nc.tensor.matmul(
                            p_n[:, MH + m, 0:B],
                            whn_sb[:, k, m * 128:(m + 1) * 128],
                            h_prev_bf[:, k, :],
                            start=(k == 0), stop=(k == KH - 1),
                        )

                s_rz = gwork.tile([128, MRZ, B], F32, tag="s_rz")
                nc.vector.tensor_add(s_rz, p_rz[:, :, 0:B], c_rz)
                g_rz = gwork.tile([128, MRZ, B], F32, tag="g_rz")
                nc.scalar.activation(
                    out=g_rz, in_=s_rz, func=mybir.ActivationFunctionType.Sigmoid)
                r = g_rz[:, 0:MH, :]
                z = g_rz[:, MH:MRZ, :]

                s_hn = gwork.tile([128, MH, B], F32, tag="s_hn")
                nc.vector.tensor_add(s_hn, p_n[:, MH:2 * MH, 0:B], bhn_bc)
                t1 = gwork.tile([128, MH, B], F32, tag="t1")
                nc.vector.tensor_mul(t1, r, s_hn)
                nc.vector.tensor_add(t1, t1, p_n[:, 0:MH, 0:B])
                nc.vector.tensor_add(t1, t1, c_n)
                n_g = gwork.tile([128, MH, B], F32, tag="n_g")
                nc.scalar.activation(
                    out=n_g, in_=t1, func=mybir.ActivationFunctionType.Tanh)

                d = gwork.tile([128, MH, B], F32, tag="d")
                nc.vector.tensor_sub(d, h_prev, n_g)
                nc.vector.tensor_mul(d, z, d)
                h_new = state.tile([128, MH, B], F32, tag="h")
                nc.vector.tensor_add(h_new, n_g, d)
                h_new_bf = state.tile([128, MH, B], GDT, tag="hbf")
                nc.vector.tensor_copy(h_new_bf, h_new)

                for m in range(ME):
                    for k in range(KH):
                        nc.tensor.matmul(
                            p_o[:, m, 0:B],
                            wo_sb[:, k, m * 128:(m + 1) * 128],
                            h_new_bf[:, k, :],
                            start=(k == 0), stop=(k == KH - 1),
                        )
                for k in range(KH):
                    nc.tensor.matmul(
                        p_o[0:1, ME, 0:B],
                        wos_sb[:, k:k + 1],
                        h_new_bf[:, k, :],
                        start=(k == 0), stop=(k == KH - 1),
                    )

                e_new = e_slot(t)
                for m in range(ME):
                    nc.scalar.activation(
                        out=e_new[:, m, :], in_=p_o[:, m, 0:B],
                        func=mybir.ActivationFunctionType.Tanh,
                        bias=bo_sb[:, m:m + 1])
                st = gwork.tile([1, B], F32, tag="st")
                nc.scalar.activation(
                    out=st, in_=p_o[0:1, ME, 0:B],
                    func=mybir.ActivationFunctionType.Sigmoid,
                    bias=bo_s_sb[0:1, 0:1])
                nc.sync.dma_start(out=stops[:, t * B:(t + 1) * B], in_=st)

                e_prev, h_prev, h_prev_bf = e_new, h_new, h_new_bf
                if (t + 1) % 8 == 0:
                    unembed_group(t // 8)

    nc.compile()
    return nc


def _prep_inputs(inputs):
    f = lambda x: np.ascontiguousarray(np.asarray(x, dtype=np.float32))
    Wi, Wh, Wp = f(inputs["Wi"]), f(inputs["Wh"]), f(inputs["Wp"])
    bi, bh, bp = f(inputs["bi"]), f(inputs["bh"]), f(inputs["bp"])
    Wo, bo = f(inputs["Wo"]), f(inputs["bo"])
    Wu, bu = f(inputs["Wu"]), f(inputs["bu"])
    W_lh, b_lh = f(inputs["W_lh"]), f(inputs["b_lh"])
    latent, start = f(inputs["latent"]), f(inputs["start"])

    import ml_dtypes
    _npdt = {"bf16": ml_dtypes.bfloat16, "f32": np.float32, "f32r": np.float32}
    gdt = _npdt[GATES_DT]
    udt = _npdt[UNEMBED_DT]

    def pb(v, slots):
        # (slots*128,) -> (128, slots, B) broadcast over batch
        return np.ascontiguousarray(
            np.repeat(v.reshape(slots, 128).T[:, :, None], B, axis=2))

    base = {
        "wrzT": np.ascontiguousarray(
            np.concatenate([Wi[:2 * H], Wh[:2 * H]], axis=1).T).astype(gdt),
        "winT": np.ascontiguousarray(Wi[2 * H:].T).astype(gdt),
        "whnT": np.ascontiguousarray(Wh[2 * H:].T).astype(gdt),
        "wpT": np.ascontiguousarray(Wp.T),
        "wlhT": np.ascontiguousarray(W_lh.T),
        "woT": np.ascontiguousarray(Wo[:E].T).astype(gdt),
        "latT": np.ascontiguousarray(latent.T),
        "e0T": np.ascontiguousarray(
            np.repeat(start[:, None], B, axis=1)).astype(gdt),
        "brz_b": pb((bi + bh + bp)[:2 * H], 16),
        "bnc_b": pb((bi + bp)[2 * H:], 8),
        "bhn_b": pb(bh[2 * H:], 8),
        "blh_b": pb(b_lh, 8),
        "bo_p": np.ascontiguousarray(bo[:E].reshape(E // 128, 128).T),
        "bo_s": bo[E:E + 1].reshape(1, 1),
        "wos_p": np.ascontiguousarray(Wo[E].reshape(H // 128, 128).T).astype(gdt),
    }
    in_maps = []
    for c in range(NCORES):
        m = dict(base)
        m["wuT"] = np.ascontiguousarray(Wu[c * VS:(c + 1) * VS].T).astype(udt)
        m["bu_b"] = np.ascontiguousarray(
            np.repeat(bu[None, c * VS:(c + 1) * VS], 128, axis=0))
        in_maps.append(m)
    return in_maps


def kernel(**inputs):
    if "nc" not in _CACHE:
        _CACHE["nc"] = build()
    nc = _CACHE["nc"]
    in_maps = _prep_inputs(inputs)
    res = run_bass_kernel_spmd(nc, in_maps, list(range(NCORES)))
    outs = [res.results[c]["logits"] for c in range(NCORES)]
    logits = np.concatenate(outs, axis=1).reshape(T, B, V).transpose(1, 0, 2)
    stops = res.results[0]["stops"].reshape(T, B).T[:, :, None]
    return np.ascontiguousarray(logits), np.ascontiguousarray(stops)


# revision 40
# speedup vs baseline: 2.1939x; 1.0415x over previous
"""Trainium2 Bass kernel for nn_Decoder_78486232367391.

GRU decoder: 63 sequential recurrence steps + a large vocab unembed.
Strategy:
  - Phase 1 (recurrence): computed redundantly on all 8 cores in
    (feature-on-partitions, batch-on-free) layout; embeddings for every
    timestep are accumulated into an internal DRAM buffer.
  - Phase 2 (unembed): vocab dimension (V=32000) sharded column-wise,
    4000 columns per core; batched over all T*B=3072 rows at full PE
    utilization. Host concatenates the 8 vocab slices.
Outputs match the jax reference: (outputs (B,T,V), stops (B,T,1)).
"""

import numpy as np

import concourse.bass as bass
import concourse.mybir as mybir
import concourse.tile as tile
from concourse import bacc
from concourse.bass_utils import run_bass_kernel_spmd

B = 48
E = 512
H = 1024
V = 32000
T = 64
NCORES = 8
VS = V // NCORES  # vocab slice per core

F32 = mybir.dt.float32
BF16 = mybir.dt.bfloat16
F32R = mybir.dt.float32r

import os as _os
# matmul precision knobs ("bf16" | "f32" | "f32r"); host prep must agree
GATES_DT = _os.environ.get("K_GATES_DT", "bf16")     # recurrence weight matmuls
UNEMBED_DT = _os.environ.get("K_UNEMBED_DT", "bf16")  # vocab projection

_DT = {"bf16": BF16, "f32": F32, "f32r": F32R}

_CACHE = {}


def _bc(dram, ap):
    """Broadcast/strided view of a DRAM tensor via an explicit access pattern."""
    a = dram[:] if not isinstance(dram, bass.AP) else dram
    return bass.AP(tensor=a.tensor, offset=a.offset, ap=ap)


def build(t_steps=T):
    nc = bacc.Bacc("TRN2")
    GDT = _DT[GATES_DT]
    UDT = _DT[UNEMBED_DT]

    def inp(name, shape, dt=F32):
        return nc.declare_dram_parameter(name, list(shape), dt, isOutput=False)

    wrzT = inp("wrzT", (E + H, 2 * H), GDT)  # [emb;h] -> r,z rows
    winT = inp("winT", (E, H), GDT)          # emb -> i_n
    whnT = inp("whnT", (H, H), GDT)          # h -> h_n
    wpT = inp("wpT", (H, 3 * H))             # latent -> peep
    wlhT = inp("wlhT", (H, H))               # latent -> hidden0
    woT = inp("woT", (H, E), GDT)            # h -> o[:E]
    wuT = inp("wuT", (E, VS), UDT)           # per-core unembed slice
    latT = inp("latT", (H, B))
    e0T = inp("e0T", (E, B), GDT)
    # biases pre-broadcast on host into on-chip layouts
    brz_b = inp("brz_b", (128, 2 * H // 128, B))   # bi+bh+bp over rz rows
    bnc_b = inp("bnc_b", (128, H // 128, B))       # bi+bp over n rows
    bhn_b = inp("bhn_b", (128, H // 128, B))       # bh over n rows
    blh_b = inp("blh_b", (128, H // 128, B))
    bo_p = inp("bo_p", (128, E // 128))
    bo_s = inp("bo_s", (1, 1))
    wos_p = inp("wos_p", (128, H // 128), GDT)     # stop-row weights per k-tile
    bu_b = inp("bu_b", (128, VS))
    assert GATES_DT == UNEMBED_DT  # ebuf feeds phase 2 directly

    logits = nc.declare_dram_parameter("logits", [t_steps * B, VS], F32, isOutput=True)
    stops = nc.declare_dram_parameter("stops", [1, t_steps * B], F32, isOutput=True)

    assert t_steps % 8 == 0  # unembed runs on groups of 8 steps (384 rows)

    KRZ = (E + H) // 128   # 12 contraction tiles for rz
    KE = E // 128          # 4
    KH = H // 128          # 8
    MRZ = 2 * H // 128     # 16 gate row tiles
    MH = H // 128          # 8
    ME = E // 128          # 4

    with tile.TileContext(nc) as tc:
        import contextlib
        with contextlib.ExitStack() as stack:
            consts = stack.enter_context(tc.tile_pool(name="consts", bufs=1))
            # broadcast bias matrices (value per (partition, slot), repeated over batch)
            bhn_bc = consts.tile([128, MH, B], F32)
            nc.sync.dma_start(out=bhn_bc, in_=bhn_b[:, :, :])
            bo_sb = consts.tile([128, ME], F32)
            nc.sync.dma_start(out=bo_sb, in_=bo_p[:, :])
            bo_s_sb = consts.tile([1, 1], F32)
            nc.sync.dma_start(out=bo_s_sb, in_=bo_s[:, :])
            c_rz = consts.tile([128, MRZ, B], F32)
            c_n = consts.tile([128, MH, B], F32)
            h0_sb = consts.tile([128, KH, B], F32)
            h0_bf = consts.tile([128, KH, B], GDT)
            # ring of the last 8 steps' embeddings, consumed by the unembed
            ering = consts.tile([128, KE, 8 * B], GDT)
            wu_sb = consts.tile([128, KE, VS], UDT)
            for k in range(KE):
                nc.sync.dma_start(out=wu_sb[:, k, :], in_=wuT[k * 128:(k + 1) * 128, :])
            bu_bc = consts.tile([128, VS], F32)
            nc.sync.dma_start(out=bu_bc, in_=bu_b[:, :])

            # ---- pre-step: peep gates (c_rz, c_n) and hidden0, streamed ----
            with tc.tile_pool(name="prew", bufs=6) as prew, \
                 tc.tile_pool(name="prec", bufs=1) as prec, \
                 tc.tile_pool(name="prepsum", bufs=1, space="PSUM") as prepsum:
                brz_bc = prec.tile([128, MRZ, B], F32)
                nc.sync.dma_start(out=brz_bc, in_=brz_b[:, :, :])
                bnc_bc = prec.tile([128, MH, B], F32)
                nc.sync.dma_start(out=bnc_bc, in_=bnc_b[:, :, :])
                blh_bc = prec.tile([128, MH, B], F32)
                nc.sync.dma_start(out=blh_bc, in_=blh_b[:, :, :])
                lat_sb = prec.tile([128, KH, B], F32)
                for k in range(KH):
                    nc.sync.dma_start(
                        out=lat_sb[:, k, :], in_=latT[k * 128:(k + 1) * 128, :])

                p_pre = prepsum.tile([128, 3 * MH, 64], F32)   # 24 slots: rz then n
                for m in range(3 * MH):
                    for k in range(KH):
                        wp_blk = prew.tile([128, 128], F32, tag="wp")
                        nc.sync.dma_start(
                            out=wp_blk,
                            in_=wpT[k * 128:(k + 1) * 128, m * 128:(m + 1) * 128])
                        nc.tensor.matmul(
                            p_pre[:, m, 0:B], wp_blk, lat_sb[:, k, :],
                            start=(k == 0), stop=(k == KH - 1),
                        )
                nc.vector.tensor_add(c_rz, p_pre[:, 0:MRZ, 0:B], brz_bc)
                nc.vector.tensor_add(c_n, p_pre[:, MRZ:3 * MH, 0:B], bnc_bc)

                p_h0 = prepsum.tile([128, MH, 64], F32)
                for m in range(MH):
                    for k in range(KH):
                        wlh_blk = prew.tile([128, 128], F32, tag="wp")
                        nc.sync.dma_start(
                            out=wlh_blk,
                            in_=wlhT[k * 128:(k + 1) * 128, m * 128:(m + 1) * 128])
                        nc.tensor.matmul(
                            p_h0[:, m, 0:B], wlh_blk, lat_sb[:, k, :],
                            start=(k == 0), stop=(k == KH - 1),
                        )
                nc.vector.tensor_add(h0_sb, p_h0[:, :, 0:B], blh_bc)
                nc.vector.tensor_copy(h0_bf, h0_sb)

                # initial embedding (start token) + stop_0 = 0
                for k in range(KE):
                    nc.sync.dma_start(
                        out=ering[:, k, 0:B], in_=e0T[k * 128:(k + 1) * 128, :])
                s0 = prec.tile([1, B], F32)
                nc.vector.memset(s0, 0.0)
                nc.sync.dma_start(out=stops[:, 0:B], in_=s0)

            # ---- recurrence weights (resident) ----
            wpool = stack.enter_context(tc.tile_pool(name="rweights", bufs=1))
            wrz_sb = wpool.tile([128, KRZ, 2 * H], GDT)
            for k in range(KRZ):
                nc.sync.dma_start(out=wrz_sb[:, k, :], in_=wrzT[k * 128:(k + 1) * 128, :])
            win_sb = wpool.tile([128, KE, H], GDT)
            for k in range(KE):
                nc.sync.dma_start(out=win_sb[:, k, :], in_=winT[k * 128:(k + 1) * 128, :])
            whn_sb = wpool.tile([128, KH, H], GDT)
            for k in range(KH):
                nc.sync.dma_start(out=whn_sb[:, k, :], in_=whnT[k * 128:(k + 1) * 128, :])
            wo_sb = wpool.tile([128, KH, E], GDT)
            for k in range(KH):
                nc.sync.dma_start(out=wo_sb[:, k, :], in_=woT[k * 128:(k + 1) * 128, :])
            wos_sb = wpool.tile([128, KH], GDT)
            nc.sync.dma_start(out=wos_sb, in_=wos_p[:, :])

            # ---- main recurrence with interleaved unembed groups ----
            state = stack.enter_context(tc.tile_pool(name="state", bufs=2))
            gwork = stack.enter_context(tc.tile_pool(name="gwork", bufs=2))
            p2out = stack.enter_context(tc.tile_pool(name="p2out", bufs=4))
            mmp = stack.enter_context(tc.tile_pool(name="mmp", bufs=1, space="PSUM"))
            mmp1 = stack.enter_context(tc.tile_pool(name="mmp1", bufs=1, space="PSUM"))
            mmp2 = stack.enter_context(tc.tile_pool(name="mmp2", bufs=2, space="PSUM"))
            p2ps = stack.enter_context(tc.tile_pool(name="p2ps", bufs=2, space="PSUM"))
            NT = (VS + 511) // 512

            def unembed_group(g):
                # steps 8g..8g+7 fill ering columns 0..383; emit 3 row tiles
                for j in range(3):
                    for n in range(NT):
                        nw = min(512, VS - n * 512)
                        p2 = p2ps.tile([128, 512], F32, tag="p2")
                        for k in range(KE):
                            nc.tensor.matmul(
                                p2[:, 0:nw],
                                ering[:, k, j * 128:(j + 1) * 128],
                                wu_sb[:, k, n * 512:n * 512 + nw],
                                start=(k == 0), stop=(k == KE - 1),
                            )
                        o_t = p2out.tile([128, 512], F32, tag="o")
                        nc.vector.tensor_add(
                            o_t[:, 0:nw], p2[:, 0:nw], bu_bc[:, n * 512:n * 512 + nw])
                        row = g * 384 + j * 128
                        nc.sync.dma_start(
                            out=logits[row:row + 128, n * 512:n * 512 + nw],
                            in_=o_t[:, 0:nw])

            def e_slot(t):
                return ering[:, :, (t % 8) * B:(t % 8 + 1) * B]

            e_prev, h_prev, h_prev_bf = e_slot(0), h0_sb, h0_bf
            for t in range(1, t_steps):
                p_rz = mmp.tile([128, MRZ, 64], F32, tag="p_rz")
                p_n = mmp1.tile([128, 2 * MH, 64], F32, tag="p_n")
                p_o = mmp2.tile([128, MH, 64], F32, tag="p_o")

                def xcat(k):
                    return e_prev[:, k, :] if k < KE else h_prev_bf[:, k - KE, :]

                for m in range(MRZ):
                    for k in range(KRZ):
                        nc.tensor.matmul(
                            p_rz[:, m, 0:B],
                            wrz_sb[:, k, m * 128:(m + 1) * 128],
                            xcat(k),
                            start=(k == 0), stop=(k == KRZ - 1),
                        )
                for m in range(MH):
                    for k in range(KE):
                        nc.tensor.matmul(
                            p_n[:, m, 0:B],
                            win_sb[:, k, m * 128:(m + 1) * 128],
                            e_prev[:, k, :],
                            start=(k == 0), stop=(k == KE - 1),
                        )
                for m in range(MH):
                    for k in range(KH):
                        nc.tensor.matmul(
                            p_n[:, MH + m, 0:B],
                            whn_sb[:, k, m * 128:(m + 1) * 128],
                            h_prev_bf[:, k, :],
                            start=(k == 0), stop=(k == KH - 1),
                        )

                s_rz = gwork.tile([128, MRZ, B], F32, tag="s_rz")
                nc.vector.tensor_add(s_rz, p_rz[:, :, 0:B], c_rz)
                g_rz = gwork.tile([128, MRZ, B], F32, tag="g_rz")
                nc.scalar.activation(
                    out=g_rz, in_=s_rz, func=mybir.ActivationFunctionType.Sigmoid)
                r = g_rz[:, 0:MH, :]
                z = g_rz[:, MH:MRZ, :]

                # off-critical-path pre-adds (independent of the sigmoid)
                s_hn = gwork.tile([128, MH, B], F32, tag="s_hn")
                nc.vector.tensor_add(s_hn, p_n[:, MH:2 * MH, 0:B], bhn_bc)
                s_in = gwork.tile([128, MH, B], F32, tag="s_in")
                nc.vector.tensor_add(s_in, p_n[:, 0:MH, 0:B], c_n)

                t1 = gwork.tile([128, MH, B], F32, tag="t1")
                nc.vector.tensor_mul(t1, r, s_hn)
                nc.vector.tensor_add(t1, t1, s_in)
                n_g = gwork.tile([128, MH, B], F32, tag="n_g")
                nc.scalar.activation(
                    out=n_g, in_=t1, func=mybir.ActivationFunctionType.Tanh)

                d = gwork.tile([128, MH, B], F32, tag="d")
                nc.vector.tensor_sub(d, h_prev, n_g)
                nc.vector.tensor_mul(d, z, d)
                # bf16 copy feeds the o-projection first (critical path);
                # the fp32 state add runs in parallel for the next step
                h_new_bf = state.tile([128, MH, B], GDT, tag="hbf")
                nc.vector.tensor_add(h_new_bf, n_g, d)
                h_new = state.tile([128, MH, B], F32, tag="h")
                nc.vector.tensor_add(h_new, n_g, d)

                for m in range(ME):
                    for k in range(KH):
                        nc.tensor.matmul(
                            p_o[:, m, 0:B],
                            wo_sb[:, k, m * 128:(m + 1) * 128],
                            h_new_bf[:, k, :],
                            start=(k == 0), stop=(k == KH - 1),
                        )
                for k in range(KH):
                    nc.tensor.matmul(
                        p_o[0:1, ME, 0:B],
                        wos_sb[:, k:k + 1],
                        h_new_bf[:, k, :],
                        start=(k == 0), stop=(k == KH - 1),
                    )

                e_new = e_slot(t)
                for m in range(ME):
                    nc.scalar.activation(
                        out=e_new[:, m, :], in_=p_o[:, m, 0:B],
                        func=mybir.ActivationFunctionType.Tanh,
                        bias=bo_sb[:, m:m + 1])
                st = gwork.tile([1, B], F32, tag="st")
                nc.scalar.activation(
                    out=st, in_=p_o[0:1, ME, 0:B],
                    func=mybir.ActivationFunctionType.Sigmoid,
                    bias=bo_s_sb[0:1, 0:1])
                nc.sync.dma_start(out=stops[:, t * B:(t + 1) * B], in_=st)

                e_prev, h_prev, h_prev_bf = e_new, h_new, h_new_bf
                if (t + 1) % 8 == 0:
                    unembed_group(t // 8)

    nc.compile()
    return nc


def _prep_inputs(inputs):
    f = lambda x: np.ascontiguousarray(np.asarray(x, dtype=np.float32))
    Wi, Wh, Wp = f(inputs["Wi"]), f(inputs["Wh"]), f(inputs["Wp"])
    bi, bh, bp = f(inputs["bi"]), f(inputs["bh"]), f(inputs["bp"])
    Wo, bo = f(inputs["Wo"]), f(inputs["bo"])
    Wu, bu = f(inputs["Wu"]), f(inputs["bu"])
    W_lh, b_lh = f(inputs["W_lh"]), f(inputs["b_lh"])
    latent, start = f(inputs["latent"]), f(inputs["start"])

    import ml_dtypes
    _npdt = {"bf16": ml_dtypes.bfloat16, "f32": np.float32, "f32r": np.float32}
    gdt = _npdt[GATES_DT]
    udt = _npdt[UNEMBED_DT]

    def pb(v, slots):
        # (slots*128,) -> (128, slots, B) broadcast over batch
        return np.ascontiguousarray(
            np.repeat(v.reshape(slots, 128).T[:, :, None], B, axis=2))

    base = {
        "wrzT": np.ascontiguousarray(
            np.concatenate([Wi[:2 * H], Wh[:2 * H]], axis=1).T).astype(gdt),
        "winT": np.ascontiguousarray(Wi[2 * H:].T).astype(gdt),
        "whnT": np.ascontiguousarray(Wh[2 * H:].T).astype(gdt),
        "wpT": np.ascontiguousarray(Wp.T),
        "wlhT": np.ascontiguousarray(W_lh.T),
        "woT": np.ascontiguousarray(Wo[:E].T).astype(gdt),
        "latT": np.ascontiguousarray(latent.T),
        "e0T": np.ascontiguousarray(
            np.repeat(start[:, None], B, axis=1)).astype(gdt),
        "brz_b": pb((bi + bh + bp)[:2 * H], 16),
        "bnc_b": pb((bi + bp)[2 * H:], 8),
        "bhn_b": pb(bh[2 * H:], 8),
        "blh_b": pb(b_lh, 8),
        "bo_p": np.ascontiguousarray(bo[:E].reshape(E // 128, 128).T),
        "bo_s": bo[E:E + 1].reshape(1, 1),
        "wos_p": np.ascontiguousarray(Wo[E].reshape(H // 128, 128).T).astype(gdt),
    }
    in_maps = []
    for c in range(NCORES):
        m = dict(base)
        m["wuT"] = np.ascontiguousarray(Wu[c * VS:(c + 1) * VS].T).astype(udt)
        m["bu_b"] = np.ascontiguousarray(
            np.repeat(bu[None, c * VS:(c + 1) * VS], 128, axis=0))
        in_maps.append(m)
    return in_maps


def kernel(**inputs):
    if "nc" not in _CACHE:
        _CACHE["nc"] = build()
    nc = _CACHE["nc"]
    in_maps = _prep_inputs(inputs)
    res = run_bass_kernel_spmd(nc, in_maps, list(range(NCORES)))
    outs = [res.results[c]["logits"] for c in range(NCORES)]
    logits = np.concatenate(outs, axis=1).reshape(T, B, V).transpose(1, 0, 2)
    stops = res.results[0]["stops"].reshape(T, B).T[:, :, None]
    return np.ascontiguousarray(logits), np.ascontiguousarray(stops)
